# revision 1
# baseline (speedup 1.0000x reference)
"""Trainium2 Bass kernel for nn_MemoryReader (retrieval_knn).

Math (per batch b):
  mk_h [h,c,n] (c=16, n=THW=8192), qk_h/qe_h [h,c,m] (m=HW=1024)
  logits[h,n,m] = (ms[n]/8) * ( sum_c mk^3*(-qe) + mk*(2*qk*qe) + (-b_sq) )
  aff = softmax over h
  mem[h,c',m] = sum_n mo[h,c',n] * aff[h,n,m]   (c'=128)
  out = concat(mem, qv)

Sharding: 8 cores = 2 batches x 4 THW-chunks (n-chunk 2048/core). Softmax is
over heads -> core-local. Readout partial-sums over n are reduced on host
during the gather (legit unshard of a contraction-sharded axis).

Device kernel per core:
  x  [33, 4*2048]  : per head [mk^3*msn; mk*msn; msn] (msn = ms/8 folded in,
                     row 32 of ones*msn folds the -b_sq term via w row 32)
  w  [33, 4*1024]  : per head [-qe; 2*qk*qe; -b_sq]
  mvt[2048, 512]   : mv chunk transposed (n on partitions for readout matmul)
  -> sim matmul (K=33, fp32r) -> exp (ACT) -> sum/recip/mul (DVE) ->
     readout matmul accumulating over the 16 n-tiles in PSUM -> mem [512,1024]
"""

import sys

sys.path.insert(0, "/opt/trn_rl_repo")

import numpy as np

import concourse.bass as bass
import concourse.tile as tile
from concourse import bacc, mybir
from concourse.bass_utils import run_bass_kernel_spmd

try:
    import ml_dtypes

    _BF16_NP = np.dtype(ml_dtypes.bfloat16)
except ImportError:  # pragma: no cover
    _BF16_NP = None

HEADS, B, CK, CV = 4, 2, 64, 512
T, H, W = 8, 32, 32
THW, HW = T * H * W, H * W          # 8192, 1024
C = CK // HEADS                      # 16
NCHUNK = THW // 4                    # 2048 n per core
NT = NCHUNK // 128                   # 16 n-tiles per core
KDIM = 2 * C + 1                     # 33

F32 = mybir.dt.float32
F32R = mybir.dt.float32r
BF16 = mybir.dt.bfloat16

# ---- tunables -------------------------------------------------------------
USE_F32R_SIM = True      # bitcast sim matmul operands to float32r (4x faster)
USE_F32R_RO = True       # same for readout matmul (only if EW_DT is f32)
EW_DT = BF16             # dtype of e/aff (softmax elementwise) + mvt
RECIP = "approx"         # "approx" (fp32 NR approx) | "plain"
# ---------------------------------------------------------------------------


def _np_dt(dt):
    return _BF16_NP if dt == BF16 else np.float32


def build_bass():
    # Bacc (not plain Bass): its compile()/finalize() pipeline legalizes
    # multi-wait instructions (TRN2 allows 1 wait/inst) via event semaphores.
    nc = bacc.Bacc(None)
    sim_dt = F32R if USE_F32R_SIM else F32
    # float32r must be produced as float32r (verifier: consumer-side bitcast
    # is rejected), so declare the DRAM + SBUF tensors with the dtype the
    # matmul consumes. Bits are identical to f32; numpy side stays float32.
    ro_dt = F32R if (EW_DT == F32 and USE_F32R_RO) else EW_DT
    # xw row-tiled layout: partitions 0-63 hold heads {0,2} (33 real rows,
    # zero-padded to 64), partitions 64-127 hold heads {1,3}. Head pair
    # (2p, 2p+1) runs as two CONCURRENT K=64 matmuls via tile_position
    # (0,0)/(64,0) -- halves sim streaming time on the PE.
    PB = NCHUNK + HW  # per-pair free block: [X 2048 | W 1024]
    xw_d = nc.dram_tensor("xw", [128, 2 * PB], sim_dt, kind="ExternalInput")
    mvt_d = nc.dram_tensor("mvt", [NCHUNK, CV], ro_dt, kind="ExternalInput")
    mem_d = nc.dram_tensor("mem", [CV, HW], F32, kind="ExternalOutput")

    Exp = mybir.ActivationFunctionType.Exp
    Copy = mybir.ActivationFunctionType.Copy

    with tile.TileContext(nc) as tc:
        with (
            tc.tile_pool(name="const", bufs=1) as constp,
            tc.tile_pool(name="simp", bufs=2, space="PSUM") as simp,
            tc.tile_pool(name="memp", bufs=1, space="PSUM") as memp,
            tc.tile_pool(name="work", bufs=6) as work,
            tc.tile_pool(name="outp", bufs=2) as outp,
        ):
            xw_sb = constp.tile([128, 2 * PB], sim_dt)
            # Interleave pair-0/pair-1 chunks (W halves first, then X
            # quarters) so BOTH pairs' first tiles arrive early — the first
            # iteration needs pr0 and pr1 data.
            for wh in range(2):
                for pr in range(2):
                    o = pr * PB + NCHUNK + wh * 512
                    nc.sync.dma_start(
                        out=xw_sb[:, o : o + 512], in_=xw_d[:, o : o + 512]
                    )
            for xh in range(4):
                for pr in range(2):
                    o = pr * PB + xh * (NCHUNK // 4)
                    nc.sync.dma_start(
                        out=xw_sb[:, o : o + NCHUNK // 4],
                        in_=xw_d[:, o : o + NCHUNK // 4],
                    )
            mvt_sb = constp.tile([128, NT * CV], ro_dt)
            for nt in range(NT):
                nc.sync.dma_start(
                    out=mvt_sb[:, nt * CV : (nt + 1) * CV],
                    in_=mvt_d[nt * 128 : (nt + 1) * 128, :],
                )

            # Heater: back-to-back dummy MMs warm the PE (HAM) before the
            # loop. Source is a memset tile (not DMA'd data) so the heater
            # runs DURING the input-DMA wait instead of after it, and the PE
            # is already at K=8/8 when the first sims arrive.
            hsrc = constp.tile([64, 768], BF16)
            nc.vector.memset(hsrc[:], 0.0)
            warm = simp.tile([128, 1024], F32, tag="sim")
            for _ in range(10):
                wmm = nc.tensor.matmul(
                    warm[:, :512],
                    lhsT=hsrc[:, 0:128],
                    rhs=hsrc[:, 128:640],
                    start=True,
                    stop=True,
                    tile_position=(0, 0),
                )
                wmm.ins.bass_priority = -100  # pin to the front of the PE queue

            for mh in range(2):
                mem_ps = memp.tile([128, 4 * 512], F32)
                for nt in range(NT):
                    # --- similarity logits: 4 heads, K=33, N=512 ---
                    simA = simp.tile([128, 1024], F32, tag="sim")
                    simB = simp.tile([128, 1024], F32, tag="sim")
                    for pr in range(2):
                        ps = simA if pr == 0 else simB
                        for half in range(2):
                            base = half * 64
                            nc.tensor.matmul(
                                ps[:, half * 512 : half * 512 + 512],
                                lhsT=xw_sb[base : base + 64,
                                           pr * PB + nt * 128 : pr * PB + nt * 128 + 128],
                                rhs=xw_sb[base : base + 64,
                                          pr * PB + NCHUNK + mh * 512 : pr * PB + NCHUNK + mh * 512 + 512],
                                start=True,
                                stop=True,
                                tile_position=(base, 0),
                            )
                    # --- softmax over heads (no max-sub: |logit| <= ~20) ---
                    e_all = work.tile([128, 2048], EW_DT, tag="e")
                    nc.scalar.activation(e_all[:, :1024], simA[:], Exp)
                    nc.scalar.activation(e_all[:, 1024:], simB[:], Exp)
                    sp = work.tile([128, 1024], EW_DT, tag="sp")
                    nc.vector.tensor_add(sp[:], e_all[:, :1024], e_all[:, 1024:])
                    s_f = work.tile([128, 512], F32, tag="S")
                    nc.gpsimd.tensor_add(s_f[:], sp[:, :512], sp[:, 512:])
                    # custom NR reciprocal writing bf16 directly (out-dtype
                    # conversion happens at the DVE write port) — saves the
                    # separate f32->bf16 cast op.
                    from concourse.dve_ops import (
                        RECIP_APPROX_FAST_CONSTS as _RC,
                        RECIPROCAL_APPROX_FAST as _RF,
                    )
                    r_use = work.tile([128, 512], EW_DT, tag="Rb")
                    nc.vector._custom_dve(
                        _RF,
                        out=r_use[:],
                        in0=s_f[:],
                        s0=_RC["s0"],
                        s1=_RC["s1"],
                        imm2=_RC["imm2"],
                    )
                    aff = work.tile([128, 4 * 512], ro_dt, tag="aff")
                    nc.vector.tensor_mul(
                        aff.rearrange("p (h m) -> p h m", h=4),
                        e_all.rearrange("p (h m) -> p h m", h=4),
                        r_use[:, None, :].to_broadcast((128, 4, 512)),
                    )
                    # --- readout: accumulate over n-tiles in PSUM ---
                    # Deprioritized (higher bass_priority = scheduled later):
                    # readouts only gate the end-of-half flush, while the next
                    # iteration's sims gate the whole softmax chain on ACT/DVE.
                    for h in range(HEADS):
                        ro = nc.tensor.matmul(
                            mem_ps[:, h * 512 : (h + 1) * 512],
                            lhsT=mvt_sb[:, nt * CV + h * 128 : nt * CV + h * 128 + 128],
                            rhs=aff[:, h * 512 : (h + 1) * 512],
                            start=(nt == 0),
                            stop=(nt == NT - 1),
                        )
                        ro.ins.bass_priority = 40
                mem_sb = outp.tile([128, 4 * 512], F32)
                for h in range(HEADS):
                    # per-head copy so each output DMA starts as soon as its
                    # slice is staged (shorter kernel tail)
                    nc.scalar.activation(
                        mem_sb[:, h * 512 : (h + 1) * 512],
                        mem_ps[:, h * 512 : (h + 1) * 512],
                        Copy,
                    )
                    nc.sync.dma_start(
                        out=mem_d[h * 128 : (h + 1) * 128, mh * 512 : (mh + 1) * 512],
                        in_=mem_sb[:, h * 512 : (h + 1) * 512],
                    )
    return nc


def host_decompose(mk, qk, ms, qe, mv):
    """Build the 8 per-core input dicts."""
    mk_f = np.asarray(mk, np.float32).reshape(B, CK, THW)
    mv_f = np.asarray(mv, np.float32).reshape(B, CV, THW)
    ms_f = np.asarray(ms, np.float32).reshape(B, THW)
    qk_h = np.asarray(qk, np.float32).reshape(B, HEADS, C, HW)
    qe_h = np.asarray(qe, np.float32).reshape(B, HEADS, C, HW)

    msn = ms_f / np.float32(np.sqrt(CK))                       # [B, THW]
    mk3 = mk_f * mk_f * mk_f                                   # [B, CK, THW]

    # w [B, 33, h, m]
    w_all = np.empty((B, KDIM, HEADS, HW), np.float32)
    w_all[:, :C] = -np.swapaxes(qe_h, 1, 2)
    w_all[:, C : 2 * C] = np.swapaxes(2.0 * qk_h * qe_h, 1, 2)
    w_all[:, 2 * C] = -np.sum(qe_h * qk_h**3, axis=2)

    # x [B, 33, h, n]
    x_all = np.empty((B, KDIM, HEADS, THW), np.float32)
    mk3_h = mk3.reshape(B, HEADS, C, THW)
    mk_h = mk_f.reshape(B, HEADS, C, THW)
    x_all[:, :C] = np.swapaxes(mk3_h, 1, 2) * msn[:, None, None, :]
    x_all[:, C : 2 * C] = np.swapaxes(mk_h, 1, 2) * msn[:, None, None, :]
    x_all[:, 2 * C] = msn[:, None, :]

    mvt_np = _np_dt(EW_DT)
    PB = NCHUNK + HW
    in_maps = []
    for core in range(8):
        b, j = core // 4, core % 4
        sl = slice(j * NCHUNK, (j + 1) * NCHUNK)
        xw = np.zeros((128, 2 * PB), np.float32)
        for pr in range(2):
            for half in range(2):
                h = 2 * pr + half
                r0 = half * 64
                xw[r0 : r0 + KDIM, pr * PB : pr * PB + NCHUNK] = x_all[b, :, h, sl]
                xw[r0 : r0 + KDIM, pr * PB + NCHUNK : (pr + 1) * PB] = w_all[b, :, h]
        mvt = np.ascontiguousarray(mv_f[b, :, sl].T).astype(mvt_np)
        in_maps.append({"xw": xw, "mvt": mvt})
    return in_maps


_NC_CACHE = None


def _get_nc():
    global _NC_CACHE
    if _NC_CACHE is None:
        nc = build_bass()
        if not nc.is_finalized():
            nc.finalize()  # Bacc compile: wait legalization etc.
        _NC_CACHE = nc
    return _NC_CACHE


def kernel(mk, qk, ms, qe, mv, qv, _trace=False, _trace_kwargs=None):
    in_maps = host_decompose(mk, qk, ms, qe, mv)
    nc = _get_nc()
    res = run_bass_kernel_spmd(
        nc, in_maps, list(range(8)), trace=_trace, **(_trace_kwargs or {})
    )
    mem = np.zeros((B, CV, HW), np.float32)
    for core in range(8):
        mem[core // 4] += res.results[core]["mem"]
    out = np.concatenate(
        [mem.reshape(B, CV, H, W), np.asarray(qv, np.float32).reshape(B, CV, H, W)],
        axis=1,
    )
    if _trace:
        return out, res
    return out



# revision 3
# speedup vs baseline: 1.1037x; 1.1037x over previous
"""Trainium2 Bass kernel for nn_MemoryReader (retrieval_knn).

Math (per batch b, with softmax over the 4 heads):
  sim_h[n,m] = msn[n] * (sum_c -qe_h*mk_h^3 + 2qk_h*qe_h*mk_h - b_h[m]),
  aff = softmax_h(sim), mem[h,c',m] = sum_n mo[h,c',n] aff[h,n,m].

Difference-softmax form (exact): with d_h = sim_h - sim_0 for h=1..3,
  r = 1/(1 + sum_h exp(d_h)),  aff_0 = r,  aff_h = exp(d_h) * r.
Only THREE exps per (n,m) instead of four; aff_0 needs no multiply.

Each d_h is one K=65 f32r matmul: rows = [mk3_h*msn; mk_h*msn;
mk3_0*msn; mk_0*msn; msn] against w rows [-qe_h; 2qk_h*qe_h; +qe_0;
-2qk_0*qe_0; (b_0-b_h)].

Sharding: 8 cores = 2 batches x 4 THW-chunks (n-chunk 2048/core). Softmax
over heads is core-local; readout partial sums over n are reduced on host.

Per-core dataflow, per (mh half of m, nt of 16 n-tiles):
  3 sim matmuls -> PSUM [128,1536] (3 banks, double-buffered = 6 banks)
  one Exp (ACT) -> e bf16 [128,1536]
  t = e1+e2 (DVE), s = (t+1)+e3 (GPSIMD STT, f32)
  r = recip_approx(s) -> bf16, written straight into the aff buffer
      (it IS head-0's affinity)
  aff_h = e_h * r (DVE, one [128,3,512] broadcast multiply)
  readout: one PSUM bank per head-pass ([128,512], 2 slots), 16 accumulating
  matmuls per pass; head 0 interleaved with the nt loop, heads 1-3 deferred
  (aff persists in SBUF) so the PE back-fills gaps and stays HAM-warm.
"""

import sys

sys.path.insert(0, "/opt/trn_rl_repo")

import numpy as np

import concourse.bass as bass
import concourse.tile as tile
from concourse import bacc, mybir
from concourse.bass_utils import run_bass_kernel_spmd

try:
    import ml_dtypes

    _BF16_NP = np.dtype(ml_dtypes.bfloat16)
except ImportError:  # pragma: no cover
    _BF16_NP = None

HEADS, B, CK, CV = 4, 2, 64, 512
T, H, W = 8, 32, 32
THW, HW = T * H * W, H * W          # 8192, 1024
C = CK // HEADS                      # 16
NCHUNK = THW // 4                    # 2048 n per core
NT = NCHUNK // 128                   # 16 n-tiles per core
KD = 4 * C + 1                       # 65 rows of the diff matmul

F32 = mybir.dt.float32
F32R = mybir.dt.float32r
BF16 = mybir.dt.bfloat16

Add = mybir.AluOpType.add


def build_bass():
    nc = bacc.Bacc(None)
    # float32r must be produced as float32r (consumer-side bitcast rejected);
    # numpy side stays float32 (identical bits).
    xs_d = nc.dram_tensor("xs", [KD, 3 * NCHUNK], F32R, kind="ExternalInput")
    ws_d = nc.dram_tensor("ws", [KD, 3 * HW], F32R, kind="ExternalInput")
    mvt_d = nc.dram_tensor("mvt", [NCHUNK, CV], BF16, kind="ExternalInput")
    mem_d = nc.dram_tensor("mem", [CV, HW], F32, kind="ExternalOutput")

    Exp = mybir.ActivationFunctionType.Exp
    Copy = mybir.ActivationFunctionType.Copy

    from concourse.dve_ops import (
        RECIP_APPROX_FAST_CONSTS as _RC,
        RECIPROCAL_APPROX_FAST as _RF,
    )

    with tile.TileContext(nc) as tc:
        with (
            tc.tile_pool(name="const", bufs=1) as constp,
            tc.tile_pool(name="simp", bufs=2, space="PSUM") as simp,
            tc.tile_pool(name="memp", bufs=2, space="PSUM") as memp,
            tc.tile_pool(name="work", bufs=4) as work,
            tc.tile_pool(name="affp", bufs=1) as affp,
            tc.tile_pool(name="outp", bufs=4) as outp,
        ):
            ws_sb = constp.tile([128, 3 * HW], F32R)
            nc.sync.dma_start(out=ws_sb[:KD, :], in_=ws_d[:, :])
            xs_sb = constp.tile([128, 3 * NCHUNK], F32R)
            # front chunks (nt 0..3) of each head first so nt=0 can start
            FR = 4 * 128
            for h in range(3):
                nc.sync.dma_start(
                    out=xs_sb[:KD, h * NCHUNK : h * NCHUNK + FR],
                    in_=xs_d[:, h * NCHUNK : h * NCHUNK + FR],
                )
            mvt_sb = constp.tile([128, NT * CV], BF16)
            for nt in range(4):
                nc.sync.dma_start(
                    out=mvt_sb[:, nt * CV : (nt + 1) * CV],
                    in_=mvt_d[nt * 128 : (nt + 1) * 128, :],
                )
            for h in range(3):
                nc.sync.dma_start(
                    out=xs_sb[:KD, h * NCHUNK + FR : (h + 1) * NCHUNK],
                    in_=xs_d[:, h * NCHUNK + FR : (h + 1) * NCHUNK],
                )
            for nt in range(4, NT):
                nc.sync.dma_start(
                    out=mvt_sb[:, nt * CV : (nt + 1) * CV],
                    in_=mvt_d[nt * 128 : (nt + 1) * 128, :],
                )

            # Heater: dummy MMs from a memset tile warm the PE (HAM) during
            # the input-DMA wait so the first sims run at K=8/8.
            hsrc = constp.tile([64, 768], BF16)
            nc.vector.memset(hsrc[:], 0.0)
            warm = simp.tile([128, 1536], F32, tag="sim")
            for _ in range(10):
                wmm = nc.tensor.matmul(
                    warm[:, :512],
                    lhsT=hsrc[:, 0:128],
                    rhs=hsrc[:, 128:640],
                    start=True,
                    stop=True,
                    tile_position=(0, 0),
                )
                wmm.ins.bass_priority = -100

            for mh in range(2):
                aff = affp.tile([128, NT * 2048], BF16, tag="aff")
                mem0 = memp.tile([128, 512], F32, tag="mem")
                for nt in range(NT):
                    st = simp.tile([128, 1536], F32, tag="sim")
                    for h in range(3):
                        nc.tensor.matmul(
                            st[:, h * 512 : (h + 1) * 512],
                            lhsT=xs_sb[:KD, h * NCHUNK + nt * 128 : h * NCHUNK + nt * 128 + 128],
                            rhs=ws_sb[:KD, h * HW + mh * 512 : h * HW + mh * 512 + 512],
                            start=True,
                            stop=True,
                        )
                    e = work.tile([128, 1536], BF16, tag="e")
                    nc.scalar.activation(e[:], st[:], Exp)
                    t = work.tile([128, 512], BF16, tag="t")
                    nc.gpsimd.tensor_add(t[:], e[:, :512], e[:, 512:1024])
                    s1 = work.tile([128, 512], F32, tag="s1")
                    nc.vector.scalar_tensor_tensor(
                        s1[:], t[:], 1.0, e[:, 1024:1536], Add, Add
                    )
                    ab = nt * 2048
                    # r = 1/(1+sum e) in bf16, written in place as aff_0
                    nc.vector._custom_dve(
                        _RF,
                        out=aff[:, ab : ab + 512],
                        in0=s1[:],
                        s0=_RC["s0"],
                        s1=_RC["s1"],
                        imm2=_RC["imm2"],
                    )
                    nc.vector.tensor_mul(
                        aff[:, ab + 512 : ab + 2048].rearrange(
                            "p (h m) -> p h m", h=3
                        ),
                        e.rearrange("p (h m) -> p h m", h=3),
                        aff[:, ab : ab + 512][:, None, :].to_broadcast(
                            (128, 3, 512)
                        ),
                    )
                    # head-0 readout rides along with the nt loop
                    ro = nc.tensor.matmul(
                        mem0[:],
                        lhsT=mvt_sb[:, nt * CV : nt * CV + 128],
                        rhs=aff[:, ab : ab + 512],
                        start=(nt == 0),
                        stop=(nt == NT - 1),
                    )
                    ro.ins.bass_priority = 40
                mem_prev = mem0
                for p in range(1, HEADS + 1):
                    # stage + DMA the finished pass while the next one runs
                    ms = outp.tile([128, 512], F32, tag="ms")
                    if p % 2:
                        nc.scalar.activation(ms[:], mem_prev[:], Copy)
                    else:
                        nc.vector.tensor_copy(ms[:], mem_prev[:])
                    nc.sync.dma_start(
                        out=mem_d[(p - 1) * 128 : p * 128, mh * 512 : (mh + 1) * 512],
                        in_=ms[:],
                    )
                    if p == HEADS:
                        break
                    mp = memp.tile([128, 512], F32, tag="mem")
                    for nt in range(NT):
                        ro = nc.tensor.matmul(
                            mp[:],
                            lhsT=mvt_sb[:, nt * CV + p * 128 : nt * CV + p * 128 + 128],
                            rhs=aff[:, nt * 2048 + p * 512 : nt * 2048 + (p + 1) * 512],
                            start=(nt == 0),
                            stop=(nt == NT - 1),
                        )
                        ro.ins.bass_priority = 50 + p
                    mem_prev = mp
    return nc


def host_decompose(mk, qk, ms, qe, mv):
    """Build the 8 per-core input dicts."""
    mk_f = np.asarray(mk, np.float32).reshape(B, CK, THW)
    mv_f = np.asarray(mv, np.float32).reshape(B, CV, THW)
    ms_f = np.asarray(ms, np.float32).reshape(B, THW)
    qk_h = np.asarray(qk, np.float32).reshape(B, HEADS, C, HW)
    qe_h = np.asarray(qe, np.float32).reshape(B, HEADS, C, HW)

    msn = ms_f / np.float32(np.sqrt(CK))                       # [B, THW]
    mk_h = mk_f.reshape(B, HEADS, C, THW)
    mk3_h = mk_h * mk_h * mk_h
    b_h = np.sum(qe_h * qk_h**3, axis=2)                       # [B, HEADS, HW]

    # xs [B, 65, 3, THW]: per diff-head (real head h+1)
    xs_all = np.empty((B, KD, 3, THW), np.float32)
    ws_all = np.empty((B, KD, 3, HW), np.float32)
    for h in range(3):
        rh = h + 1
        xs_all[:, 0:C, h] = mk3_h[:, rh]
        xs_all[:, C : 2 * C, h] = mk_h[:, rh]
        xs_all[:, 2 * C : 3 * C, h] = mk3_h[:, 0]
        xs_all[:, 3 * C : 4 * C, h] = mk_h[:, 0]
        xs_all[:, 4 * C, h] = 1.0
        ws_all[:, 0:C, h] = -qe_h[:, rh]
        ws_all[:, C : 2 * C, h] = 2.0 * qk_h[:, rh] * qe_h[:, rh]
        ws_all[:, 2 * C : 3 * C, h] = qe_h[:, 0]
        ws_all[:, 3 * C : 4 * C, h] = -2.0 * qk_h[:, 0] * qe_h[:, 0]
        ws_all[:, 4 * C, h] = b_h[:, 0] - b_h[:, rh]
    xs_all *= msn[:, None, None, :]

    in_maps = []
    for core in range(8):
        b, j = core // 4, core % 4
        sl = slice(j * NCHUNK, (j + 1) * NCHUNK)
        xs = np.ascontiguousarray(
            xs_all[b, :, :, sl].reshape(KD, 3 * NCHUNK)
        )
        ws = np.ascontiguousarray(ws_all[b].reshape(KD, 3 * HW))
        mvt = np.ascontiguousarray(mv_f[b, :, sl].T).astype(_BF16_NP)
        in_maps.append({"xs": xs, "ws": ws, "mvt": mvt})
    return in_maps


_NC_CACHE = None


def _get_nc():
    global _NC_CACHE
    if _NC_CACHE is None:
        nc = build_bass()
        if not nc.is_finalized():
            nc.finalize()
        _NC_CACHE = nc
    return _NC_CACHE


def kernel(mk, qk, ms, qe, mv, qv, _trace=False, _trace_kwargs=None):
    in_maps = host_decompose(mk, qk, ms, qe, mv)
    nc = _get_nc()
    res = run_bass_kernel_spmd(
        nc, in_maps, list(range(8)), trace=_trace, **(_trace_kwargs or {})
    )
    mem = np.zeros((B, CV, HW), np.float32)
    for core in range(8):
        mem[core // 4] += res.results[core]["mem"]
    out = np.concatenate(
        [mem.reshape(B, CV, H, W), np.asarray(qv, np.float32).reshape(B, CV, H, W)],
        axis=1,
    )
    if _trace:
        return out, res
    return out


# revision 4
# speedup vs baseline: 1.1300x; 1.0239x over previous
"""Trainium2 Bass kernel for nn_MemoryReader (retrieval_knn).

Math (per batch b, with softmax over the 4 heads):
  sim_h[n,m] = msn[n] * (sum_c -qe_h*mk_h^3 + 2qk_h*qe_h*mk_h - b_h[m]),
  aff = softmax_h(sim), mem[h,c',m] = sum_n mo[h,c',n] aff[h,n,m].

Difference-softmax form (exact): with d_h = sim_h - sim_0 for h=1..3,
  r = 1/(1 + sum_h exp(d_h)),  aff_0 = r,  aff_h = exp(d_h) * r.
Only THREE exps per (n,m) instead of four; aff_0 needs no multiply.

Each d_h is one K=65 f32r matmul: rows = [mk3_h*msn; mk_h*msn;
mk3_0*msn; mk_0*msn; msn] against w rows [-qe_h; 2qk_h*qe_h; +qe_0;
-2qk_0*qe_0; (b_0-b_h)].

Sharding: 8 cores = 2 batches x 4 THW-chunks (n-chunk 2048/core). Softmax
over heads is core-local; readout partial sums over n are reduced on host.

Per-core dataflow, per (mh half of m, nt of 16 n-tiles):
  3 sim matmuls -> PSUM [128,1536] (3 banks, double-buffered = 6 banks)
  one Exp (ACT) -> e bf16 [128,1536]
  t = e1+e2 (DVE), s = (t+1)+e3 (GPSIMD STT, f32)
  r = recip_approx(s) -> bf16, written straight into the aff buffer
      (it IS head-0's affinity)
  aff_h = e_h * r (DVE, one [128,3,512] broadcast multiply)
  readout: one PSUM bank per head-pass ([128,512], 2 slots), 16 accumulating
  matmuls per pass; head 0 interleaved with the nt loop, heads 1-3 deferred
  (aff persists in SBUF) so the PE back-fills gaps and stays HAM-warm.
"""

import sys

sys.path.insert(0, "/opt/trn_rl_repo")

import numpy as np

import concourse.bass as bass
import concourse.tile as tile
from concourse import bacc, mybir
from concourse.bass_utils import run_bass_kernel_spmd

try:
    import ml_dtypes

    _BF16_NP = np.dtype(ml_dtypes.bfloat16)
except ImportError:  # pragma: no cover
    _BF16_NP = None

HEADS, B, CK, CV = 4, 2, 64, 512
T, H, W = 8, 32, 32
THW, HW = T * H * W, H * W          # 8192, 1024
C = CK // HEADS                      # 16
NCHUNK = THW // 4                    # 2048 n per core
NT = NCHUNK // 128                   # 16 n-tiles per core
KD = 4 * C + 1                       # 65 rows of the diff matmul

F32 = mybir.dt.float32
F32R = mybir.dt.float32r
BF16 = mybir.dt.bfloat16

Add = mybir.AluOpType.add


def build_bass():
    nc = bacc.Bacc(None)
    # float32r must be produced as float32r (consumer-side bitcast rejected);
    # numpy side stays float32 (identical bits).
    xs_d = nc.dram_tensor("xs", [KD, 3 * NCHUNK], BF16, kind="ExternalInput")
    ws_d = nc.dram_tensor("ws", [KD, 3 * HW], BF16, kind="ExternalInput")
    mvt_d = nc.dram_tensor("mvt", [NCHUNK, CV], BF16, kind="ExternalInput")
    mem_d = nc.dram_tensor("mem", [CV, HW], F32, kind="ExternalOutput")

    Exp = mybir.ActivationFunctionType.Exp
    Copy = mybir.ActivationFunctionType.Copy

    from concourse.dve_ops import (
        RECIP_APPROX_FAST_CONSTS as _RC,
        RECIPROCAL_APPROX_FAST as _RF,
    )

    with tile.TileContext(nc) as tc:
        with (
            tc.tile_pool(name="const", bufs=1) as constp,
            tc.tile_pool(name="simp", bufs=2, space="PSUM") as simp,
            tc.tile_pool(name="memp", bufs=2, space="PSUM") as memp,
            tc.tile_pool(name="work", bufs=4) as work,
            tc.tile_pool(name="affp", bufs=1) as affp,
            tc.tile_pool(name="outp", bufs=4) as outp,
        ):
            ws_sb = constp.tile([128, 3 * HW], BF16)
            nc.sync.dma_start(out=ws_sb[:KD, :], in_=ws_d[:, :])
            xs_sb = constp.tile([128, 3 * NCHUNK], BF16)
            # front chunks (nt 0..3) of each head first so nt=0 can start
            FR = 4 * 128
            for h in range(3):
                nc.sync.dma_start(
                    out=xs_sb[:KD, h * NCHUNK : h * NCHUNK + FR],
                    in_=xs_d[:, h * NCHUNK : h * NCHUNK + FR],
                )
            mvt_sb = constp.tile([128, NT * CV], BF16)
            for nt in range(4):
                nc.sync.dma_start(
                    out=mvt_sb[:, nt * CV : (nt + 1) * CV],
                    in_=mvt_d[nt * 128 : (nt + 1) * 128, :],
                )
            for h in range(3):
                nc.sync.dma_start(
                    out=xs_sb[:KD, h * NCHUNK + FR : (h + 1) * NCHUNK],
                    in_=xs_d[:, h * NCHUNK + FR : (h + 1) * NCHUNK],
                )
            for nt in range(4, NT):
                nc.sync.dma_start(
                    out=mvt_sb[:, nt * CV : (nt + 1) * CV],
                    in_=mvt_d[nt * 128 : (nt + 1) * 128, :],
                )

            # Heater: dummy MMs from a memset tile warm the PE (HAM) during
            # the input-DMA wait so the first sims run at K=8/8.
            hsrc = constp.tile([64, 768], BF16)
            nc.vector.memset(hsrc[:], 0.0)
            warm = simp.tile([128, 1536], F32, tag="sim")
            for _ in range(10):
                wmm = nc.tensor.matmul(
                    warm[:, :512],
                    lhsT=hsrc[:, 0:128],
                    rhs=hsrc[:, 128:640],
                    start=True,
                    stop=True,
                    tile_position=(0, 0),
                )
                wmm.ins.bass_priority = -100

            for mh in range(2):
                aff = affp.tile([128, NT * 2048], BF16, tag="aff")
                mem0 = memp.tile([128, 512], F32, tag="mem")
                for nt in range(NT):
                    st = simp.tile([128, 1536], F32, tag="sim")
                    for h in range(3):
                        nc.tensor.matmul(
                            st[:, h * 512 : (h + 1) * 512],
                            lhsT=xs_sb[:KD, h * NCHUNK + nt * 128 : h * NCHUNK + nt * 128 + 128],
                            rhs=ws_sb[:KD, h * HW + mh * 512 : h * HW + mh * 512 + 512],
                            start=True,
                            stop=True,
                        )
                    e = work.tile([128, 1536], BF16, tag="e")
                    nc.scalar.activation(e[:], st[:], Exp)
                    t = work.tile([128, 512], BF16, tag="t")
                    nc.gpsimd.tensor_add(t[:], e[:, :512], e[:, 512:1024])
                    s1 = work.tile([128, 512], BF16, tag="s1")
                    nc.vector.scalar_tensor_tensor(
                        s1[:], t[:], 1.0, e[:, 1024:1536], Add, Add
                    )
                    ab = nt * 2048
                    # r = 1/(1+sum e) in bf16, written in place as aff_0
                    nc.vector._custom_dve(
                        _RF,
                        out=aff[:, ab : ab + 512],
                        in0=s1[:],
                        s0=_RC["s0"],
                        s1=_RC["s1"],
                        imm2=_RC["imm2"],
                    )
                    nc.vector.tensor_mul(
                        aff[:, ab + 512 : ab + 2048].rearrange(
                            "p (h m) -> p h m", h=3
                        ),
                        e.rearrange("p (h m) -> p h m", h=3),
                        aff[:, ab : ab + 512][:, None, :].to_broadcast(
                            (128, 3, 512)
                        ),
                    )
                    # head-0 readout rides along with the nt loop
                    ro = nc.tensor.matmul(
                        mem0[:],
                        lhsT=mvt_sb[:, nt * CV : nt * CV + 128],
                        rhs=aff[:, ab : ab + 512],
                        start=(nt == 0),
                        stop=(nt == NT - 1),
                    )
                    ro.ins.bass_priority = 40
                mem_prev = mem0
                for p in range(1, HEADS + 1):
                    # stage + DMA the finished pass while the next one runs
                    ms = outp.tile([128, 512], F32, tag="ms")
                    if p % 2:
                        nc.scalar.activation(ms[:], mem_prev[:], Copy)
                    else:
                        nc.vector.tensor_copy(ms[:], mem_prev[:])
                    nc.sync.dma_start(
                        out=mem_d[(p - 1) * 128 : p * 128, mh * 512 : (mh + 1) * 512],
                        in_=ms[:],
                    )
                    if p == HEADS:
                        break
                    mp = memp.tile([128, 512], F32, tag="mem")
                    for nt in range(NT):
                        ro = nc.tensor.matmul(
                            mp[:],
                            lhsT=mvt_sb[:, nt * CV + p * 128 : nt * CV + p * 128 + 128],
                            rhs=aff[:, nt * 2048 + p * 512 : nt * 2048 + (p + 1) * 512],
                            start=(nt == 0),
                            stop=(nt == NT - 1),
                        )
                        ro.ins.bass_priority = 50 + p
                    mem_prev = mp
    return nc


def host_decompose(mk, qk, ms, qe, mv):
    """Build the 8 per-core input dicts."""
    mk_f = np.asarray(mk, np.float32).reshape(B, CK, THW)
    mv_f = np.asarray(mv, np.float32).reshape(B, CV, THW)
    ms_f = np.asarray(ms, np.float32).reshape(B, THW)
    qk_h = np.asarray(qk, np.float32).reshape(B, HEADS, C, HW)
    qe_h = np.asarray(qe, np.float32).reshape(B, HEADS, C, HW)

    msn = ms_f / np.float32(np.sqrt(CK))                       # [B, THW]
    mk_h = mk_f.reshape(B, HEADS, C, THW)
    mk3_h = mk_h * mk_h * mk_h
    b_h = np.sum(qe_h * qk_h**3, axis=2)                       # [B, HEADS, HW]

    # xs [B, 65, 3, THW]: per diff-head (real head h+1)
    xs_all = np.empty((B, KD, 3, THW), np.float32)
    ws_all = np.empty((B, KD, 3, HW), np.float32)
    for h in range(3):
        rh = h + 1
        xs_all[:, 0:C, h] = mk3_h[:, rh]
        xs_all[:, C : 2 * C, h] = mk_h[:, rh]
        xs_all[:, 2 * C : 3 * C, h] = mk3_h[:, 0]
        xs_all[:, 3 * C : 4 * C, h] = mk_h[:, 0]
        xs_all[:, 4 * C, h] = 1.0
        ws_all[:, 0:C, h] = -qe_h[:, rh]
        ws_all[:, C : 2 * C, h] = 2.0 * qk_h[:, rh] * qe_h[:, rh]
        ws_all[:, 2 * C : 3 * C, h] = qe_h[:, 0]
        ws_all[:, 3 * C : 4 * C, h] = -2.0 * qk_h[:, 0] * qe_h[:, 0]
        ws_all[:, 4 * C, h] = b_h[:, 0] - b_h[:, rh]
    xs_all *= msn[:, None, None, :]

    in_maps = []
    for core in range(8):
        b, j = core // 4, core % 4
        sl = slice(j * NCHUNK, (j + 1) * NCHUNK)
        xs = np.ascontiguousarray(
            xs_all[b, :, :, sl].reshape(KD, 3 * NCHUNK)
        ).astype(_BF16_NP)
        ws = np.ascontiguousarray(ws_all[b].reshape(KD, 3 * HW)).astype(_BF16_NP)
        mvt = np.ascontiguousarray(mv_f[b, :, sl].T).astype(_BF16_NP)
        in_maps.append({"xs": xs, "ws": ws, "mvt": mvt})
    return in_maps


_NC_CACHE = None


def _get_nc():
    global _NC_CACHE
    if _NC_CACHE is None:
        nc = build_bass()
        if not nc.is_finalized():
            nc.finalize()
        _NC_CACHE = nc
    return _NC_CACHE


def kernel(mk, qk, ms, qe, mv, qv, _trace=False, _trace_kwargs=None):
    in_maps = host_decompose(mk, qk, ms, qe, mv)
    nc = _get_nc()
    res = run_bass_kernel_spmd(
        nc, in_maps, list(range(8)), trace=_trace, **(_trace_kwargs or {})
    )
    mem = np.zeros((B, CV, HW), np.float32)
    for core in range(8):
        mem[core // 4] += res.results[core]["mem"]
    out = np.concatenate(
        [mem.reshape(B, CV, H, W), np.asarray(qv, np.float32).reshape(B, CV, H, W)],
        axis=1,
    )
    if _trace:
        return out, res
    return out
